# revision 37
# baseline (speedup 1.0000x reference)
"""Multi-head attention with sparse mask + post-softmax group_prob modulation.

B=8, S=1024, D=1024, H=16 heads (DK=64). Sharded batch-parallel across 8
NeuronCores (one batch element per core). Inputs are marshaled on host into
feature-major (transposed) layouts so every matmul contraction runs with the
contract dim on SBUF partitions; q/k/v and the q/k/v projection weights are
cast to bf16 host-side (halves DMA traffic, full-rate PE either way):

  per core (batch b):
    qT/kT    = x[b].T bf16, qt-major halves [256, 4S] (DMA-panel blocked)
    vT       = v[b].T bf16, kc-major blocks [S, S]
    MT       = ((mask|I)!=0)[b].T [S, S]  bf16   (k-major 0/1 mask)
    gT       = group_prob[b].T   [S, S]   bf16   (k-major)

  device pipeline:
    V        = vT-chunk-stationary x Wv moving  -> [s, dv] bf16 (+bv via DVE)
    per head pair j (heads 2j, 2j+1 in partition halves of dk-chunk j):
      KT/QT[j] = Wq/Wk-chunk-stationary matmuls -> [dk, s] fp32 (+bias, ACT)
      scores.T[k,q] psum = KT_h^T @ QT_h (fp32r, K=64, tile-positioned)
      e_m    = exp(psum / sqrt(dk)) -> bf16 (ACT reads PSUM, scale folds 1/8)
      e_mask = e_m * MT (DVE bf16), e_g = e_mask * gT (DVE bf16)
      denom[q] = ones^T @ e_mask (PE partition-reduce, accum over k chunks)
      x.T psum[dv,q] = V-slice-stationary @ e_g (col-packed head pair)
      X.T = psum * R -> qs-major DRAM scratch (R = recips broadcast, K=1 MM;
            last pair stays in SBUF; next pair's projections are emitted
            ahead of this normalize/store chain to keep PE fed)
    out[q,:] = X.T-chunk-stationary @ Wo + bo -> DMA out (natural layout)
"""

import os
from contextlib import ExitStack

import ml_dtypes
import numpy as np

import concourse.bacc as bacc
import concourse.bass as bass
import concourse.mybir as mybir
import concourse.tile as tile

B, S, D, H = 8, 1024, 1024, 16
DK = D // H  # 64
NCH = S // 128  # 8 chunks of 128
F32 = mybir.dt.float32
F32R = mybir.dt.float32r
BF16 = mybir.dt.bfloat16
AF = mybir.ActivationFunctionType
ALU = mybir.AluOpType

_CACHE = {}


def r(ap):
    """view fp32 AP as float32r for full-rate matmul"""
    return ap.bitcast(F32R)


def emit_kernel(ctx: ExitStack, tc: tile.TileContext, io: dict):
    nc = tc.nc
    qT, kT, vT = io["qT"], io["kT"], io["vT"]
    MT, gT = io["MT"], io["gT"]
    Wq, Wk, Wv, Wo = io["Wq"], io["Wk"], io["Wv"], io["Wo"]
    BQ, BK = io["BQ"], io["BK"]
    BV, BO = io["BV"], io["BO"]
    out = io["out"]

    # ---------------- pools ----------------
    res = ctx.enter_context(tc.tile_pool(name="res", bufs=1))
    qkw = ctx.enter_context(tc.tile_pool(name="qkw", bufs=2))
    qkt = ctx.enter_context(tc.tile_pool(name="qkt", bufs=2))
    em_pool = ctx.enter_context(tc.tile_pool(name="em", bufs=3))
    emask_pool = ctx.enter_context(tc.tile_pool(name="emask", bufs=3))
    eg_pool = ctx.enter_context(tc.tile_pool(name="eg", bufs=3))
    small = ctx.enter_context(tc.tile_pool(name="small", bufs=1))
    outp = ctx.enter_context(tc.tile_pool(name="outp", bufs=2))
    instream = ctx.enter_context(tc.tile_pool(name="instream", bufs=2))
    dram = ctx.enter_context(tc.tile_pool(name="dram", bufs=1, space="DRAM"))

    psum_s = ctx.enter_context(tc.tile_pool(name="ps_s", bufs=2, space="PSUM"))
    psum_x = ctx.enter_context(tc.tile_pool(name="ps_x", bufs=2, space="PSUM"))
    psum_d = ctx.enter_context(tc.tile_pool(name="ps_d", bufs=1, space="PSUM"))
    psum_bc = ctx.enter_context(tc.tile_pool(name="ps_bc", bufs=1, space="PSUM"))

    # ---------------- constants ----------------
    ones_col = small.tile([128, 1], BF16)  # denominator stationary
    nc.gpsimd.memset(ones_col[:], 1.0)
    ones_row = small.tile([1, 64], BF16)  # recip broadcast stationary (K=1)
    nc.gpsimd.memset(ones_row[:], 1.0)
    bq_sb = small.tile([128, NCH], F32)  # column ck = bias chunk ck
    bk_sb = small.tile([128, NCH], F32)

    qscale = 1.0 / float(np.sqrt(DK))  # folded into exp's activation scale

    # ------------- input DMA, ordered by first PE use ----------------------
    # vT/Wv feed the V phase immediately; qT/kT feed j=0 projections right
    # after (own SBUF slots so they stream during the V phase); w_t j=0 then
    # M/G (first scores) then Wo (tail) follow in the SP queue.
    # vT_res col kc*S + c*128 + o = value.T[c*128+p, kc*128+o] (kc-major blocks)
    # Wv_res col dt*4096 + c*512 + o = Wv[c*128+p, dt*512+o]   (dt-major halves)
    vT_res = res.tile([128, NCH * S], BF16, tag="big_a")
    Wv_res = res.tile([128, NCH * D], BF16)
    BV_sb = small.tile([128, D], F32, tag="bias_vo")
    for kc in range(NCH):
        nc.sync.dma_start(
            vT_res[:, kc * S : (kc + 1) * S], vT[kc * 128 : (kc + 1) * 128, :]
        )
        if kc == 0:  # first moving panel, then off-critical-path biases
            nc.sync.dma_start(Wv_res[:, 0 : 4 * D], Wv[0:128, :])
            nc.sync.dma_start(BV_sb[:], BV[:, :])
            nc.sync.dma_start(bq_sb[:], BQ[:, :])
            nc.sync.dma_start(bk_sb[:], BK[:, :])
    nc.sync.dma_start(Wv_res[:, 4 * D : 8 * D], Wv[128:256, :])
    # qT/kT: qt-major halves, col qt*4096 + c*512 + o = q.T[c*128+p, qt*512+o]
    qT_res = res.tile([128, NCH * S], BF16)
    kT_res = res.tile([128, NCH * S], BF16)
    for qt in range(2):
        nc.sync.dma_start(
            qT_res[:, qt * 4 * S : (qt + 1) * 4 * S], qT[qt * 128 : qt * 128 + 128, :]
        )
        nc.sync.dma_start(
            kT_res[:, qt * 4 * S : (qt + 1) * 4 * S], kT[qt * 128 : qt * 128 + 128, :]
        )

    # ---------------- V = value @ Wv + bv  -> bf16, natural [s, dv] ---------
    V_sb = res.tile([128, NCH * D], BF16)  # col block kc -> V[128*kc:+128, :]
    for dt in range(2):
        for kcp in range(0, NCH, 2):
            ps = psum_s.tile([128, 1024], F32, tag="s", name="ps_v")
            for ki in range(2):
                kc = kcp + ki
                for c in range(NCH):
                    nc.tensor.matmul(
                        ps[:, ki * 512 : (ki + 1) * 512],
                        vT_res[:, kc * S + c * 128 : kc * S + c * 128 + 128],
                        Wv_res[:, dt * 4 * D + c * 512 : dt * 4 * D + (c + 1) * 512],
                        start=(c == 0),
                        stop=(c == NCH - 1),
                    )
                nc.vector.tensor_add(
                    V_sb[:, kc * D + dt * 512 : kc * D + (dt + 1) * 512],
                    ps[:, ki * 512 : (ki + 1) * 512],
                    BV_sb[:, dt * 512 : (dt + 1) * 512],
                )

    # ---- masks/group probs (host-precomputed bf16), then Wo for the tail ---
    M_sb = res.tile([128, NCH * S], BF16)  # col block kc -> M.T[128kc:+128, :]
    G_sb = res.tile([128, NCH * S], BF16)
    # first mask/G chunks (j=0 kc=0 scores), then j=0 projection weights
    nc.sync.dma_start(M_sb[:, 0:S], MT[0:128, :])
    nc.sync.dma_start(G_sb[:, 0:S], gT[0:128, :])
    w0_t = [
        qkw.tile([128, 128 * NCH], BF16, tag="w", name=f"w0_{wi}") for wi in range(2)
    ]
    for wi, W in ((0, Wq), (1, Wk)):
        for c in range(NCH):
            nc.sync.dma_start(
                w0_t[wi][:, c * 128 : (c + 1) * 128],
                W[c * 128 : (c + 1) * 128, 0:128],
            )
    for kc in range(1, NCH):
        nc.sync.dma_start(M_sb[:, kc * S : (kc + 1) * S], MT[kc * 128 : (kc + 1) * 128, :])
        nc.sync.dma_start(G_sb[:, kc * S : (kc + 1) * S], gT[kc * 128 : (kc + 1) * 128, :])
    Wo_res = res.tile([128, NCH * D], F32R, tag="big_a")  # reuses vT slot
    for c in range(NCH):
        nc.sync.dma_start(Wo_res[:, c * D : (c + 1) * D], Wo[c * 128 : (c + 1) * 128, :])
    BO_sb = small.tile([128, D], F32, tag="bias_vo")  # reuses BV slot (V done)
    nc.sync.dma_start(BO_sb[:], BO[:, :])

    # ---------------- attention per head pair -------------------------------
    # X.T scratch, qs-major: XT_d[qs][:, c*128:+128] = x.T of pair c, q-block qs
    XT_d = dram.tile([NCH, 128, S], F32R)
    recip_pool = ctx.enter_context(tc.tile_pool(name="recip", bufs=1))

    def emit_proj(j):
        """project QT[j], KT[j]: [128 dk, S] (tiles double-buffer across j)"""
        QT_t = qkt.tile([128, S], F32R, tag="QT", name="QT_t")
        KT_t = qkt.tile([128, S], F32R, tag="KT", name="KT_t")
        for wi, (W, src_res, dst, bias_sb) in enumerate(
            (
                (Wq, qT_res, QT_t, bq_sb),
                (Wk, kT_res, KT_t, bk_sb),
            )
        ):
            if j == 0:
                w_t = w0_t[wi]
            else:
                w_t = qkw.tile([128, 128 * NCH], BF16, tag="w", name="w_t")
                for c in range(NCH):
                    nc.sync.dma_start(
                        w_t[:, c * 128 : (c + 1) * 128],
                        W[c * 128 : (c + 1) * 128, j * 128 : (j + 1) * 128],
                    )
            ps = psum_s.tile([128, 1024], F32, tag="s", name="ps_p")
            for qt in range(2):
                for c in range(NCH):
                    nc.tensor.matmul(
                        ps[:, qt * 512 : (qt + 1) * 512],
                        w_t[:, c * 128 : (c + 1) * 128],
                        src_res[:, qt * 4 * S + c * 512 : qt * 4 * S + (c + 1) * 512],
                        start=(c == 0),
                        stop=(c == NCH - 1),
                    )
                # per-half ACT so kc=0 scores start after the qt=0 halves
                nc.scalar.activation(
                    dst[:, qt * 512 : (qt + 1) * 512],
                    ps[:, qt * 512 : (qt + 1) * 512],
                    AF.Identity,
                    bias=bias_sb[:, j : j + 1],
                )
        return QT_t, KT_t

    nxt = emit_proj(0)
    xt_last = None
    for j in range(NCH):  # head pair j = heads 2j, 2j+1
        QT_t, KT_t = nxt

        # --- scores + exp + mask/G muls + denom + PV ---
        # single denominator bank: row 64*qt+32*h holds denom for (qt, head h)
        ps_dd = psum_d.tile([97, 512], F32, name="psdd", tag="psdd")
        ps_xx = [
            psum_x.tile([128, 512], F32, name="psxx", tag="psxx") for _ in range(2)
        ]
        for kc in range(NCH):
            em_t = [None, None]
            for h in range(2):  # pass 1: both heads' scores + exp
                em = em_pool.tile([128, S], BF16, tag="em")
                em_t[h] = em
                ps = psum_s.tile([128, 1024], F32, tag="s", name="ps_s")
                for qt in range(2):
                    nc.tensor.matmul(
                        ps[:, qt * 512 : (qt + 1) * 512],
                        r(KT_t[64 * h : 64 * h + 64, kc * 128 : (kc + 1) * 128]),
                        r(QT_t[64 * h : 64 * h + 64, qt * 512 : (qt + 1) * 512]),
                        start=True,
                        stop=True,
                        tile_position=(64 * h, 0),
                        skip_group_check=True,
                    )
                # e_m = exp(scores/sqrt(dk)), unmasked (|s| is small)
                nc.scalar.activation(em[:], ps[:], AF.Exp, scale=qscale)
            for h in range(2):  # pass 2: masks, denominators, PV
                em = em_t[h]
                # masked exponentials: e_mask = e_m * M, e_g = e_mask * G
                emask = emask_pool.tile([128, S], BF16, tag="emask")
                nc.vector.tensor_mul(emask[:], em[:], M_sb[:, kc * S : (kc + 1) * S])
                eg = eg_pool.tile([128, S], BF16, tag="eg")
                nc.vector.tensor_mul(eg[:], emask[:], G_sb[:, kc * S : (kc + 1) * S])
                hd = 64 * (2 * j + h)
                for qt in range(2):
                    # denominator: ones^T @ e_mask accumulated over kc
                    row = 64 * qt + 32 * h
                    nc.tensor.matmul(
                        ps_dd[row : row + 1, :],
                        ones_col[:, 0:1],
                        emask[:, qt * 512 : (qt + 1) * 512],
                        start=(kc == 0),
                        stop=(kc == NCH - 1),
                        tile_position=(0, row),
                        skip_group_check=True,
                    )
                    # PV: col-packed pair, accumulate over kc
                    nc.tensor.matmul(
                        ps_xx[qt][64 * h : 64 * h + 64, :],
                        V_sb[:, kc * D + hd : kc * D + hd + 64],
                        eg[:, qt * 512 : (qt + 1) * 512],
                        start=(kc == 0),
                        stop=(kc == NCH - 1),
                        tile_position=(0, 64 * h),
                        skip_group_check=True,
                    )
        # next head pair's projections overlap this pair's normalize/store
        if j < NCH - 1:
            nxt = emit_proj(j + 1)

        # reciprocal of denominators (rows 0 and 32 of ps_dd)
        recip_t = [
            recip_pool.tile([1, S], BF16, name=f"recip{h}", tag=f"recip{h}")
            for h in range(2)
        ]
        with nc.allow_low_precision(reason="bf16 recip feeds bf16 matmul"):
            for qt in range(2):
                for h in range(2):
                    row = 64 * qt + 32 * h
                    nc.vector.reciprocal(
                        recip_t[h][0:1, qt * 512 : (qt + 1) * 512],
                        ps_dd[row : row + 1, :],
                    )
        # R = broadcast recip rows via K=1 matmul, evict, X.T = x * R -> DRAM
        xt_t = outp.tile([128, S], F32R, tag="xt")
        for qt in range(2):
            ps_r = psum_bc.tile([128, 512], F32, tag="ps")
            for h in range(2):
                nc.tensor.matmul(
                    ps_r[64 * h : 64 * h + 64, :],
                    ones_row[0:1, 0:64],
                    recip_t[h][0:1, qt * 512 : (qt + 1) * 512],
                    start=True,
                    stop=True,
                    tile_position=(0, 64 * h),
                    skip_group_check=True,
                )
            r_sb = outp.tile([128, 512], F32, tag="rsb")
            nc.scalar.activation(r_sb[:], ps_r[:], AF.Copy, bias=0.0)
            nc.vector.tensor_mul(
                xt_t[:, qt * 512 : (qt + 1) * 512], ps_xx[qt][:], r_sb[:]
            )
        if j == NCH - 1:
            xt_last = xt_t  # last pair read from SBUF by the out projection
        else:
            for qs in range(NCH):
                nc.sync.dma_start(
                    XT_d[qs, :, j * 128 : (j + 1) * 128],
                    xt_t[:, qs * 128 : (qs + 1) * 128],
                )

    # ---------------- out = X @ Wo + bo -------------------------------------
    for qs in range(NCH):
        xin = instream.tile([128, 128 * (NCH - 1)], F32R, tag="xin")
        nc.sync.dma_start(xin[:], XT_d[qs, :, 0 : 128 * (NCH - 1)])
        o_sb = outp.tile([128, D], F32, tag="osb")
        ps = psum_s.tile([128, 1024], F32, tag="s", name="ps_o")
        for dt in range(2):
            for c in range(NCH):
                lhs = (
                    xin[:, c * 128 : (c + 1) * 128]
                    if c < NCH - 1
                    else xt_last[:, qs * 128 : (qs + 1) * 128]
                )
                nc.tensor.matmul(
                    ps[:, dt * 512 : (dt + 1) * 512],
                    r(lhs),
                    r(Wo_res[:, c * D + dt * 512 : c * D + (dt + 1) * 512]),
                    start=(c == 0),
                    stop=(c == NCH - 1),
                )
        nc.vector.tensor_add(o_sb[:], ps[:], BO_sb[:])
        nc.sync.dma_start(out[qs * 128 : (qs + 1) * 128, :], o_sb[:])


def build_module():
    if "nc" in _CACHE:
        return _CACHE["nc"], _CACHE["io"]
    nc = bacc.Bacc(
        "TRN2", target_bir_lowering=False, debug=False, enable_asserts=False
    )
    io = {}
    for name in ("vT", "gT", "MT"):
        io[name] = nc.dram_tensor(name, [S, S], BF16, kind="ExternalInput").ap()
    for name in ("qT", "kT"):
        io[name] = nc.dram_tensor(name, [256, 4 * S], BF16, kind="ExternalInput").ap()
    io["Wv"] = nc.dram_tensor("Wv", [256, 4 * D], BF16, kind="ExternalInput").ap()
    for name in ("Wq", "Wk"):
        io[name] = nc.dram_tensor(name, [D, D], BF16, kind="ExternalInput").ap()
    io["Wo"] = nc.dram_tensor("Wo", [D, D], F32R, kind="ExternalInput").ap()
    for name in ("BQ", "BK"):
        io[name] = nc.dram_tensor(name, [128, NCH], F32, kind="ExternalInput").ap()
    for name in ("BV", "BO"):
        io[name] = nc.dram_tensor(name, [128, D], F32, kind="ExternalInput").ap()
    io["out"] = nc.dram_tensor("out", [S, D], F32, kind="ExternalOutput").ap()

    with tile.TileContext(nc) as tc:
        with ExitStack() as ctx:
            emit_kernel(ctx, tc, io)
    nc.compile()
    _CACHE["nc"] = nc
    _CACHE["io"] = io
    return nc, io


def make_in_maps(**inputs):
    f32 = np.float32
    bf16 = ml_dtypes.bfloat16
    def halfmajor(a):
        # [D, N] -> [256, 4N]: row h*128+p, col c*512+o = a[c*128+p, h*512+o]
        n = a.shape[1]
        return np.ascontiguousarray(
            a.reshape(NCH, 128, 2, n // 2).transpose(2, 1, 0, 3).reshape(256, 4 * n)
        )

    shared = {
        "Wq": np.ascontiguousarray(inputs["Wq"], f32).astype(bf16),
        "Wk": np.ascontiguousarray(inputs["Wk"], f32).astype(bf16),
        "Wv": halfmajor(np.asarray(inputs["Wv"], f32).astype(bf16)),
        "Wo": np.ascontiguousarray(inputs["Wo"], f32),
        "BQ": np.ascontiguousarray(
            np.reshape(np.asarray(inputs["bq"], f32), (NCH, 128)).T
        ),
        "BK": np.ascontiguousarray(
            np.reshape(np.asarray(inputs["bk"], f32), (NCH, 128)).T
        ),
        "BV": np.ascontiguousarray(
            np.tile(np.reshape(inputs["bv"], (1, D)), (128, 1)), f32
        ),
        "BO": np.ascontiguousarray(
            np.tile(np.reshape(inputs["bo"], (1, D)), (128, 1)), f32
        ),
    }
    q, k, v = (np.asarray(inputs[n], f32) for n in ("query", "key", "value"))
    gp = np.asarray(inputs["group_prob"], f32)
    mk = np.asarray(inputs["mask"], np.int32)
    eye = np.eye(S, dtype=np.int32)
    in_maps = []
    for b in range(B):
        m = dict(shared)
        m["qT"] = halfmajor(q[b].T.astype(bf16))
        m["kT"] = halfmajor(k[b].T.astype(bf16))
        # kc-major blocks: row kc*128+p, col c*128+o = v.T[c*128+p, kc*128+o]
        m["vT"] = np.ascontiguousarray(
            v[b].T.astype(bf16)
            .reshape(NCH, 128, NCH, 128)
            .transpose(2, 1, 0, 3)
            .reshape(S, S)
        )
        m["gT"] = np.ascontiguousarray(gp[b].T.astype(bf16))
        m["MT"] = np.ascontiguousarray(
            ((mk[b] | eye) != 0).T.astype(bf16)
        )
        in_maps.append(m)
    return in_maps


def kernel(**inputs) -> np.ndarray:
    from concourse.bass_utils import run_bass_kernel_spmd

    nc, _ = build_module()
    in_maps = make_in_maps(**inputs)
    trace = bool(int(os.environ.get("KERNEL_TRACE", "0")))
    res = run_bass_kernel_spmd(nc, in_maps, core_ids=list(range(B)), trace=trace)
    _CACHE["last_result"] = res
    return np.stack([res.results[b]["out"] for b in range(B)], axis=0)
